# revision 1
# baseline (speedup 1.0000x reference)
"""Trainium2 Bass kernel for block-diagonal sparse attention (8 NeuronCores SPMD).

Problem: nn_AttentionHead (N=4096, DIM_IN=512, DQ=DK=128, 16 graphs of 256 nodes).
  q = x@Wq.T+bq; k = x@Wk.T+bk; v = x@Wv.T+bv
  a = where(block, qk/sqrt(dq), 0) + b + c; masked-softmax over block-diagonal
  out = (softmax(a)*keep) @ v

Key structural facts exploited:
  - Everything off the block diagonal is masked to -1e6 -> exp underflows to 0,
    so only the 16 diagonal 256x256 tiles of b/c/sparse_mask matter. The host
    slices exactly those blocks, cutting HBM traffic from ~200MB to ~3.5MB/core.
  - Graphs are independent -> rows shard 2-graphs-per-core across 8 cores with
    zero cross-core communication (weights replicated).
  - softmax(a)[r] = exp(a[r])/sum(exp(a[r])): |a| <~ 15 so no max-subtraction is
    needed in f32; masked entries get exp(a-100) which underflows vs kept terms.
  - The denominator is obtained for free by appending a ones-column to v in the
    PV matmul (column 128 of the PSUM accumulates sum_j e[r,j]).

Layout: q/k projections are computed transposed (d on partitions) straight from
x.T; scores are computed transposed (j on partitions) so the probability matrix
comes out in exactly the layout the PV matmul needs as its stationary operand
(no on-chip transpose of e). v is computed directly in natural layout (rows on
partitions) using x.T slices as the stationary operand; its bias lands via a
rank-1 (K=1) ones x bias^T matmul into the same PSUM accumulation group.

All DMAs ride the sync HWDGE ring (fast path); f32 -> bf16 casts happen on-chip
on DVE/ACT (the gpsimd SWDGE casting-DMA path measured ~4x slower end-to-end).
"""

import math

import numpy as np

import concourse.bass as bass
import concourse.mybir as mybir
import concourse.tile as tile
from concourse import bacc
from concourse.bass_utils import run_bass_kernel_spmd

# -------- problem constants (hardcoded per spec) --------
N = 4096
DIN = 512
DQ = 128           # == DK
NG = 16            # number of graphs
G = N // NG        # 256 nodes per graph
NCORES = 8
RPC = N // NCORES  # 512 rows per core
GPC = NG // NCORES  # 2 graphs per core
NT = RPC // 128    # 4 row-tiles of 128 per core
KO = DIN // 128    # 4 contraction tiles for the projections
VA = DQ + 1        # v augmented with a ones column (denominator trick)
SCALE = 1.0 / math.sqrt(DQ)
SENT = 100.0       # additive mask sentinel; exp bias of -SENT cancels it

F32 = mybir.dt.float32
BF16 = mybir.dt.bfloat16
I32 = mybir.dt.int32

ACT = mybir.ActivationFunctionType
ALU = mybir.AluOpType

_CACHE: dict = {}


def build_nc() -> bass.Bass:
    """Build the per-core Bass graph (identical on all 8 cores)."""
    nc = bacc.Bacc(
        "TRN2",
        target_bir_lowering=False,
        debug=False,
        enable_asserts=False,
        num_devices=NCORES,
    )
    xT_d = nc.dram_tensor("xh", [128, KO, RPC], F32, kind="ExternalInput").ap()
    w_d = nc.dram_tensor("wh", [128, 3, KO, DQ], F32, kind="ExternalInput").ap()
    bia_d = nc.dram_tensor("bias", [DQ, 3], F32, kind="ExternalInput").ap()
    biar_d = nc.dram_tensor("biasr", [1, 3, DQ], F32, kind="ExternalInput").ap()
    bc_d = nc.dram_tensor("bch", [128, 2, NT, G], F32, kind="ExternalInput").ap()
    md_d = nc.dram_tensor("mdh", [128, NT, G], I32, kind="ExternalInput").ap()
    out_d = nc.dram_tensor("out", [RPC, DQ], F32, kind="ExternalOutput").ap()

    with tile.TileContext(nc) as tc:
        with (
            tc.tile_pool(name="const", bufs=1) as cpool,
            tc.tile_pool(name="work", bufs=3) as wpool,
            tc.tile_pool(name="et", bufs=4) as epool,
            tc.tile_pool(name="ps_proj", bufs=2, space="PSUM") as pp,
            tc.tile_pool(name="ps_warm", bufs=1, space="PSUM") as pw,
            tc.tile_pool(name="ps_v", bufs=1, space="PSUM") as pvp,
            tc.tile_pool(name="ps_s", bufs=2, space="PSUM") as ps,
            tc.tile_pool(name="ps_o", bufs=2, space="PSUM") as po,
        ):
            # ---- input DMAs: split across both HWDGE rings (sync + scalar),
            # host arrays already in partition-major layout (128 descriptors
            # per DMA, contiguous multi-KB per partition)
            xT_f = cpool.tile([128, KO, RPC], F32)  # [din%128, din//128, r]
            nc.sync.dma_start(xT_f[:], xT_d)
            bc = cpool.tile([128, 2, NT, G], F32)  # [j%128, b|c, j//128, r]
            nc.sync.dma_start(bc[:], bc_d)
            md = cpool.tile([128, NT, G], I32)
            nc.sync.dma_start(md[:], md_d)
            bia = cpool.tile([128, 3], F32)  # [d, qkv]
            nc.scalar.dma_start(bia[:], bia_d)
            biar = cpool.tile([1, 3, DQ], F32)  # row layout for the v bias
            nc.scalar.dma_start(biar[:], biar_d)
            w_f = cpool.tile([128, 3, KO, DQ], F32)  # [din%128, qkv, din//128, d]
            nc.scalar.dma_start(w_f[:], w_d)

            # ---- PE HAM warmup: dummy matmuls while the DMAs stream, so the
            # real matmuls run at 2.4 GHz instead of the cold 1.2 GHz
            warm_sb = cpool.tile([128, RPC], BF16)
            nc.vector.memset(warm_sb[:], 1.0)
            for _ in range(7):
                wp = pw.tile([128, RPC], F32, tag="warm")
                nc.tensor.matmul(
                    wp[:], lhsT=warm_sb[:, 0:128], rhs=warm_sb[:],
                    start=True, stop=True,
                )

            # ---- on-chip f32 -> bf16 casts, split across DVE and ACT ----
            xT = cpool.tile([128, KO, RPC], BF16)
            for ko in range(KO):
                if ko % 2 == 0:
                    nc.vector.tensor_copy(out=xT[:, ko, :], in_=xT_f[:, ko, :])
                else:
                    nc.scalar.copy(xT[:, ko, :], xT_f[:, ko, :])
            w = cpool.tile([128, 3, KO, DQ], BF16)
            for s in range(3):
                if s % 2 == 0:
                    nc.scalar.copy(w[:, s, :, :], w_f[:, s, :, :])
                else:
                    nc.vector.tensor_copy(out=w[:, s, :, :], in_=w_f[:, s, :, :])
            biar_b = cpool.tile([1, 3, DQ], BF16)
            nc.vector.tensor_copy(out=biar_b[:], in_=biar[:])

            # small constants
            bqs = cpool.tile([128, 1], F32)  # bq / sqrt(dq)
            nc.vector.tensor_scalar_mul(bqs[:], bia[:, 0:1], SCALE)
            ones_b = cpool.tile([1, 128], BF16)  # rank-1 bias lhsT
            nc.vector.memset(ones_b[:], 1.0)

            # ---- q/k projections, transposed: pT[d, r] = (x @ W_s.T).T ----
            def proj(s):
                p = pp.tile([128, RPC], F32, tag="proj")
                for ko in range(KO):
                    nc.tensor.matmul(
                        p[:],
                        lhsT=w[:, s, ko, :],
                        rhs=xT[:, ko, :],
                        start=(ko == 0),
                        stop=(ko == KO - 1),
                    )
                return p

            qT = cpool.tile([128, RPC], BF16)
            pq = proj(0)
            nc.scalar.activation(qT[:], pq[:], ACT.Identity, bias=bqs[:], scale=SCALE)
            kT = cpool.tile([128, RPC], BF16)
            pk = proj(1)
            nc.scalar.activation(kT[:], pk[:], ACT.Identity, bias=bia[:, 1:2])

            # ---- scores (transposed), bias+mask, exp ----
            ets = []
            for t in range(NT):
                g, jb = divmod(t, 2)
                sp = ps.tile([128, G], F32, tag="s")
                nc.tensor.matmul(
                    sp[:],
                    lhsT=kT[:, g * G + jb * 128: g * G + jb * 128 + 128],
                    rhs=qT[:, g * G:(g + 1) * G],
                    start=True,
                    stop=True,
                )
                a = wpool.tile([128, G], F32, tag="a")
                nc.vector.tensor_tensor(a[:], sp[:], bc[:, 0, t, :], ALU.add)
                nc.vector.tensor_tensor(a[:], a[:], bc[:, 1, t, :], ALU.add)
                # e_raw = exp(a) unmasked (|a| small enough for f32/bf16),
                # then zero the masked entries by multiplying with the 0/1
                # mask on the otherwise-idle gpsimd engine. Identical to the
                # reference softmax(keep-masked) since masked entries also
                # drop out of the denominator (ones-column of the PV matmul).
                eraw = epool.tile([128, G], BF16, tag="eraw")
                nc.scalar.activation(eraw[:], a[:], ACT.Exp)
                et = epool.tile([128, G], BF16, tag="et")
                nc.gpsimd.tensor_tensor(et[:], eraw[:], md[:, t, :], ALU.mult)
                ets.append(et)

            # ---- v in natural layout (bf16), ones column, rank-1 bias ----
            vna = cpool.tile([128, NT, VA], BF16)  # [j%128, j//128, d | 1]
            nc.vector.memset(vna[:, :, DQ:VA], 1.0)
            for jt in range(NT):
                pv = pvp.tile([128, DQ], F32, tag="vn")
                for ko in range(KO):
                    nc.tensor.matmul(
                        pv[:],
                        lhsT=xT[:, ko, jt * 128:(jt + 1) * 128],
                        rhs=w[:, 2, ko, :],
                        start=(ko == 0),
                        stop=False,
                    )
                nc.tensor.matmul(
                    pv[:], lhsT=ones_b[:], rhs=biar_b[:, 2, :],
                    start=False, stop=True,
                )
                nc.vector.tensor_copy(out=vna[:, jt, 0:DQ], in_=pv[:])

            # ---- PV matmul (+denominator via ones column), normalize ----
            out_sb = cpool.tile([128, NT, DQ], F32)
            out_r = out_d.rearrange("(t p) d -> p t d", p=128)
            for g in range(GPC):
                for rb in range(2):
                    t = 2 * g + rb
                    op = po.tile([128, VA], F32, tag="o")
                    for jb in range(2):
                        nc.tensor.matmul(
                            op[:],
                            lhsT=ets[2 * g + jb][:, rb * 128:(rb + 1) * 128],
                            rhs=vna[:, 2 * g + jb, :],
                            start=(jb == 0),
                            stop=(jb == 1),
                        )
                    rec = wpool.tile([128, 1], F32, tag="rec")
                    nc.vector.reciprocal(rec[:], op[:, DQ:VA])
                    if rb == 0:
                        nc.scalar.activation(
                            out_sb[:, t, :], op[:, 0:DQ], ACT.Copy, scale=rec[:]
                        )
                    else:
                        nc.vector.tensor_scalar_mul(
                            out_sb[:, t, :], op[:, 0:DQ], rec[:]
                        )
                # per-graph output DMA so graph 0's store overlaps graph 1
                nc.sync.dma_start(
                    out_r[:, 2 * g:2 * g + 2, :], out_sb[:, 2 * g:2 * g + 2, :]
                )
    nc.compile()
    return nc


def get_nc() -> bass.Bass:
    if "nc" not in _CACHE:
        _CACHE["nc"] = build_nc()
    return _CACHE["nc"]


def make_in_maps(x, b, c, ptr, sparse_mask, Wq, bq, Wk, bk, Wv, bv):
    """Host-side sharding: slice the block-diagonal and transpose per layout."""
    x = np.asarray(x, dtype=np.float32)
    b = np.asarray(b, dtype=np.float32)
    c = np.asarray(c, dtype=np.float32)
    ptr = np.asarray(ptr)
    sparse_mask = np.asarray(sparse_mask, dtype=np.int32)
    w = np.ascontiguousarray(
        np.stack([np.asarray(Wq).T, np.asarray(Wk).T, np.asarray(Wv).T])
    ).astype(np.float32)  # [3, DIN, DQ]
    bias = np.ascontiguousarray(
        np.stack([np.asarray(bq), np.asarray(bk), np.asarray(bv)], axis=1)
    ).astype(np.float32)  # [DQ, 3]
    biasr = np.ascontiguousarray(
        np.stack([np.asarray(bq), np.asarray(bk), np.asarray(bv)], axis=0)[None]
    ).astype(np.float32)  # [1, 3, DQ]

    assert np.array_equal(
        np.asarray(ptr).ravel(), np.arange(NG + 1) * G
    ), "kernel compiled for uniform 256-node graphs"

    # weights in partition-major layout: wh[p, s, o, d] = W_s.T[o*128+p, d]
    wh = np.ascontiguousarray(
        w.reshape(3, KO, 128, DQ).transpose(2, 0, 1, 3)
    )  # [128, 3, KO, DQ]

    in_maps = []
    for i in range(NCORES):
        lo = i * RPC
        xT = np.ascontiguousarray(x[lo:lo + RPC].T)  # [DIN, RPC]
        # xh[p, o, r] = xT[o*128+p, r]
        xh = np.ascontiguousarray(xT.reshape(KO, 128, RPC).transpose(1, 0, 2))
        bds, cds, mds = [], [], []
        for gl in range(GPC):
            blk = slice(lo + gl * G, lo + (gl + 1) * G)
            bds.append(b[blk, blk].T)
            cds.append(c[blk, blk].T)
            mds.append(sparse_mask[blk, blk].T)
        bcT = np.stack([np.concatenate(bds, 0), np.concatenate(cds, 0)])
        # bch[p, s, t, r] = bcT[s, t*128+p, r]
        bch = np.ascontiguousarray(
            bcT.reshape(2, NT, 128, G).transpose(2, 0, 1, 3)
        ).astype(np.float32)  # [128, 2, NT, G]
        mdT = np.concatenate(mds, 0)
        mdh = np.ascontiguousarray(
            mdT.reshape(NT, 128, G).transpose(1, 0, 2)
        ).astype(np.int32)  # [128, NT, G]
        in_maps.append(
            {"xh": xh, "wh": wh, "bias": bias, "biasr": biasr,
             "bch": bch, "mdh": mdh}
        )
    return in_maps


def run(inputs: dict, trace: bool = False):
    """Run on all 8 cores; returns (full_output, BassKernelResults)."""
    nc = get_nc()
    in_maps = make_in_maps(**inputs)
    res = run_bass_kernel_spmd(
        nc, in_maps, core_ids=list(range(NCORES)), trace=trace
    )
    out = np.concatenate([r["out"] for r in res.results], axis=0)
    return out.astype(np.float32), res


def kernel(**inputs) -> np.ndarray:
    out, _ = run(inputs, trace=False)
    return out



# revision 2
# speedup vs baseline: 1.3566x; 1.3566x over previous
"""Trainium2 Bass kernel for block-diagonal sparse attention (8 NeuronCores SPMD).

Problem: nn_AttentionHead (N=4096, DIM_IN=512, DQ=DK=128, 16 graphs of 256 nodes).
  q = x@Wq.T+bq; k = x@Wk.T+bk; v = x@Wv.T+bv
  a = where(block, qk/sqrt(dq), 0) + b + c; masked-softmax over block-diagonal
  out = (softmax(a)*keep) @ v

Key structural facts exploited:
  - Everything off the block diagonal is masked to -1e6 -> exp underflows to 0,
    so only the 16 diagonal 256x256 tiles of b/c/sparse_mask matter.
  - Graphs are independent -> rows shard 2-graphs-per-core across 8 cores with
    zero cross-core communication (weights replicated).
  - exp(qk+b+c)*mask == exp(qk) * (exp(b+c)*mask): the host precomputes
    E = exp(b+c)*mask once (bf16), so the device does one ACT exp straight off
    the score PSUM and one bf16 multiply -- no mask DMA, no bias adds.
  - The denominator is obtained for free by appending a ones-column to v in the
    PV matmul (column 128 of the PSUM accumulates sum_j e[r,j]).
  - All tensors are cast to bf16 on the host: HBM traffic drops from ~3.7MB to
    ~1.3MB per core and no on-chip casts are needed. 1/sqrt(dq) is folded into
    Wq host-side.

Layout: q/k projections are computed transposed (d on partitions) straight from
x.T; scores are computed transposed (j on partitions) so the probability matrix
comes out in exactly the layout the PV matmul needs as its stationary operand.
v is computed in natural layout using x.T slices as the stationary operand; its
bias lands via a rank-1 ones x bias^T matmul in the same PSUM group.

Output is written bf16 in [128, NT, DQ] partition-major layout (contiguous 1KB
DMA descriptors) and un-permuted/upcast on the host.
"""

import math

import numpy as np
import ml_dtypes

import concourse.bass as bass
import concourse.mybir as mybir
import concourse.tile as tile
from concourse import bacc
from concourse.bass_utils import run_bass_kernel_spmd

# -------- problem constants (hardcoded per spec) --------
N = 4096
DIN = 512
DQ = 128           # == DK
NG = 16            # number of graphs
G = N // NG        # 256 nodes per graph
NCORES = 8
RPC = N // NCORES  # 512 rows per core
GPC = NG // NCORES  # 2 graphs per core
NT = RPC // 128    # 4 row-tiles of 128 per core
KO = DIN // 128    # 4 contraction tiles for the projections
VA = DQ + 1        # v augmented with a ones column (denominator trick)
SCALE = 1.0 / math.sqrt(DQ)
NWARM = 5          # PE HAM warmup matmuls

F32 = mybir.dt.float32
BF16 = mybir.dt.bfloat16

ACT = mybir.ActivationFunctionType
ALU = mybir.AluOpType

BF = ml_dtypes.bfloat16

_CACHE: dict = {}


def build_nc() -> bass.Bass:
    """Build the per-core Bass graph (identical on all 8 cores)."""
    nc = bacc.Bacc(
        "TRN2",
        target_bir_lowering=False,
        debug=False,
        enable_asserts=False,
        num_devices=NCORES,
    )
    xT_d = nc.dram_tensor("xh", [128, KO, RPC], BF16, kind="ExternalInput").ap()
    w_d = nc.dram_tensor("wh", [128, 3, KO, DQ], BF16, kind="ExternalInput").ap()
    bia_d = nc.dram_tensor("bias", [DQ, 3], F32, kind="ExternalInput").ap()
    biar_d = nc.dram_tensor("biasr", [1, 3, DQ], BF16, kind="ExternalInput").ap()
    eh_d = nc.dram_tensor("eh", [128, NT, G], BF16, kind="ExternalInput").ap()
    out_d = nc.dram_tensor("out", [128, NT, DQ], BF16, kind="ExternalOutput").ap()

    with tile.TileContext(nc) as tc:
        with (
            tc.tile_pool(name="const", bufs=1) as cpool,
            tc.tile_pool(name="work", bufs=4) as wpool,
            tc.tile_pool(name="et", bufs=4) as epool,
            tc.tile_pool(name="ps_proj", bufs=2, space="PSUM") as pp,
            tc.tile_pool(name="ps_v", bufs=2, space="PSUM") as pvp,
            tc.tile_pool(name="ps_s", bufs=2, space="PSUM") as ps,
            tc.tile_pool(name="ps_o", bufs=2, space="PSUM") as po,
        ):
            # ---- input DMAs, split across both HWDGE rings; host arrays are
            # already bf16 and partition-major (contiguous KBs per partition)
            xT = cpool.tile([128, KO, RPC], BF16)  # [din%128, din//128, r]
            nc.sync.dma_start(xT[:, 0:2, :], xT_d[:, 0:2, :])
            nc.sync.dma_start(xT[:, 2:4, :], xT_d[:, 2:4, :])
            w = cpool.tile([128, 3, KO, DQ], BF16)  # [din%128, qkv, din//128, d]
            nc.scalar.dma_start(w[:], w_d)
            bia = cpool.tile([128, 3], F32)  # [d, qkv]; q column pre-scaled
            nc.scalar.dma_start(bia[:], bia_d)
            biar = cpool.tile([1, 3, DQ], BF16)  # row layout for the v bias
            nc.scalar.dma_start(biar[:], biar_d)
            eh = cpool.tile([128, NT, G], BF16)  # exp(b+c)*mask, transposed
            nc.scalar.dma_start(eh[:], eh_d)

            # small constants
            warm = cpool.tile([128, RPC], BF16)
            nc.vector.memset(warm[:], 1.0)
            ones_b = cpool.tile([1, 128], BF16)  # rank-1 bias lhsT
            nc.vector.memset(ones_b[:], 1.0)
            vna = cpool.tile([128, NT, VA], BF16)  # [j%128, j//128, d | 1]
            nc.vector.memset(vna[:, :, DQ:VA], 1.0)

            # ---- PE HAM warmup: dummy matmuls while the DMAs stream ----
            for _ in range(NWARM):
                wp = pp.tile([128, RPC], F32, tag="proj")
                nc.tensor.matmul(
                    wp[:], lhsT=warm[:, 0:128], rhs=warm[:],
                    start=True, stop=True,
                )

            # ---- q/k projections, transposed: pT[d, r] = (x @ W_s.T).T ----
            def proj(s):
                p = pp.tile([128, RPC], F32, tag="proj")
                for ko in range(KO):
                    nc.tensor.matmul(
                        p[:],
                        lhsT=w[:, s, ko, :],
                        rhs=xT[:, ko, :],
                        start=(ko == 0),
                        stop=(ko == KO - 1),
                    )
                return p

            # evacuate in graph-halves so scores for graph 0 start early;
            # q on DVE, k on ACT to run both engines in parallel
            pq = proj(0)
            qT = cpool.tile([128, RPC], BF16)
            nc.vector.tensor_scalar_add(qT[:, 0:G], pq[:, 0:G], bia[:, 0:1])
            nc.vector.tensor_scalar_add(qT[:, G:RPC], pq[:, G:RPC], bia[:, 0:1])
            pk = proj(1)
            kT = cpool.tile([128, RPC], BF16)
            nc.scalar.activation(kT[:, 0:G], pk[:, 0:G], ACT.Identity, bias=bia[:, 1:2])
            nc.scalar.activation(kT[:, G:RPC], pk[:, G:RPC], ACT.Identity, bias=bia[:, 1:2])

            # ---- v in natural layout (bf16), ones column, rank-1 bias ----
            for jt in range(NT):
                pv = pvp.tile([128, DQ], F32, tag="vn")
                for ko in range(KO):
                    nc.tensor.matmul(
                        pv[:],
                        lhsT=xT[:, ko, jt * 128:(jt + 1) * 128],
                        rhs=w[:, 2, ko, :],
                        start=(ko == 0),
                        stop=False,
                    )
                nc.tensor.matmul(
                    pv[:], lhsT=ones_b[:], rhs=biar[:, 2, :],
                    start=False, stop=True,
                )
                if jt < 2:
                    nc.vector.tensor_copy(out=vna[:, jt, 0:DQ], in_=pv[:])
                else:
                    nc.scalar.copy(vna[:, jt, 0:DQ], pv[:])

            # ---- scores (transposed) -> exp -> * E ----
            ets = []
            for t in range(NT):
                g = t // 2
                sp = ps.tile([128, G], F32, tag="s")
                nc.tensor.matmul(
                    sp[:],
                    lhsT=kT[:, t * 128:(t + 1) * 128],
                    rhs=qT[:, g * G:(g + 1) * G],
                    start=True,
                    stop=True,
                )
                eq = epool.tile([128, G], BF16, tag="eq")
                nc.scalar.activation(eq[:], sp[:], ACT.Exp)
                et = epool.tile([128, G], BF16, tag="et")
                if t < 2:
                    nc.gpsimd.tensor_tensor(et[:], eq[:], eh[:, t, :], ALU.mult)
                else:
                    nc.vector.tensor_tensor(et[:], eq[:], eh[:, t, :], ALU.mult)
                ets.append(et)

            # ---- PV matmul (+denominator via ones column), normalize ----
            out_sb = cpool.tile([128, NT, DQ], BF16)
            for g in range(GPC):
                for rb in range(2):
                    t = 2 * g + rb
                    op = po.tile([128, VA], F32, tag="o")
                    for jb in range(2):
                        nc.tensor.matmul(
                            op[:],
                            lhsT=ets[2 * g + jb][:, rb * 128:(rb + 1) * 128],
                            rhs=vna[:, 2 * g + jb, :],
                            start=(jb == 0),
                            stop=(jb == 1),
                        )
                    rec = wpool.tile([128, 1], F32, tag="rec")
                    nc.vector.reciprocal(rec[:], op[:, DQ:VA])
                    if rb == 0:
                        nc.scalar.activation(
                            out_sb[:, t, :], op[:, 0:DQ], ACT.Copy, scale=rec[:]
                        )
                    else:
                        nc.vector.tensor_scalar_mul(
                            out_sb[:, t, :], op[:, 0:DQ], rec[:]
                        )
                # per-graph output DMA on alternating rings
                if g == 0:
                    nc.sync.dma_start(out_d[:, 0:2, :], out_sb[:, 0:2, :])
                else:
                    nc.scalar.dma_start(out_d[:, 2:4, :], out_sb[:, 2:4, :])
    nc.compile()
    return nc


def get_nc() -> bass.Bass:
    if "nc" not in _CACHE:
        _CACHE["nc"] = build_nc()
    return _CACHE["nc"]


def make_in_maps(x, b, c, ptr, sparse_mask, Wq, bq, Wk, bk, Wv, bv):
    """Host-side sharding: slice the block-diagonal, precompute exp(b+c)*mask,
    cast everything to bf16, transpose to partition-major layouts."""
    x = np.asarray(x, dtype=np.float32)
    b = np.asarray(b, dtype=np.float32)
    c = np.asarray(c, dtype=np.float32)
    ptr = np.asarray(ptr)
    mask = np.asarray(sparse_mask).astype(np.float32)
    # fold 1/sqrt(dq) into Wq/bq so scores come out pre-scaled
    w3 = np.stack(
        [np.asarray(Wq).T * SCALE, np.asarray(Wk).T, np.asarray(Wv).T]
    ).astype(np.float32)  # [3, DIN, DQ]
    bias = np.ascontiguousarray(
        np.stack(
            [np.asarray(bq) * SCALE, np.asarray(bk), np.asarray(bv)], axis=1
        )
    ).astype(np.float32)  # [DQ, 3]
    biasr = np.ascontiguousarray(
        np.stack([np.asarray(bq), np.asarray(bk), np.asarray(bv)], axis=0)[None]
    ).astype(BF)  # [1, 3, DQ]

    assert np.array_equal(
        np.asarray(ptr).ravel(), np.arange(NG + 1) * G
    ), "kernel compiled for uniform 256-node graphs"

    # weights in partition-major layout: wh[p, s, o, d] = W_s.T[o*128+p, d]
    wh = np.ascontiguousarray(
        w3.reshape(3, KO, 128, DQ).transpose(2, 0, 1, 3)
    ).astype(BF)  # [128, 3, KO, DQ]

    in_maps = []
    for i in range(NCORES):
        lo = i * RPC
        xT = x[lo:lo + RPC].T  # [DIN, RPC]
        xh = np.ascontiguousarray(
            xT.reshape(KO, 128, RPC).transpose(1, 0, 2)
        ).astype(BF)  # [128, KO, RPC]
        eds = []
        for gl in range(GPC):
            blk = slice(lo + gl * G, lo + (gl + 1) * G)
            e = np.exp(b[blk, blk] + c[blk, blk]) * mask[blk, blk]
            eds.append(e.T)  # [j, r]
        edT = np.concatenate(eds, 0)  # [RPC, G]
        ehh = np.ascontiguousarray(
            edT.reshape(NT, 128, G).transpose(1, 0, 2)
        ).astype(BF)  # [128, NT, G]
        in_maps.append(
            {"xh": xh, "wh": wh, "bias": bias, "biasr": biasr, "eh": ehh}
        )
    return in_maps


def run(inputs: dict, trace: bool = False):
    """Run on all 8 cores; returns (full_output, BassKernelResults)."""
    nc = get_nc()
    in_maps = make_in_maps(**inputs)
    res = run_bass_kernel_spmd(
        nc, in_maps, core_ids=list(range(NCORES)), trace=trace
    )
    outs = []
    for r in res.results:
        o = np.asarray(r["out"]).astype(np.float32)  # [128, NT, DQ]
        outs.append(o.transpose(1, 0, 2).reshape(RPC, DQ))
    out = np.concatenate(outs, axis=0)
    return out, res


def kernel(**inputs) -> np.ndarray:
    out, _ = run(inputs, trace=False)
    return out


# revision 4
# speedup vs baseline: 1.4344x; 1.0573x over previous
"""Trainium2 Bass kernel for block-diagonal sparse attention (8 NeuronCores SPMD).

Problem: nn_AttentionHead (N=4096, DIM_IN=512, DQ=DK=128, 16 graphs of 256 nodes).
  q = x@Wq.T+bq; k = x@Wk.T+bk; v = x@Wv.T+bv
  a = where(block, qk/sqrt(dq), 0) + b + c; masked-softmax over block-diagonal
  out = (softmax(a)*keep) @ v

Key structural facts exploited:
  - Only the 16 diagonal 256x256 tiles of b/c/sparse_mask matter; the host
    slices them, combines bcm = b+c (masked entries -> -200 so exp gives 0),
    casts to bf16. HBM traffic is ~1.3MB/core instead of ~200MB.
  - Graphs are independent -> 2 graphs per core across 8 cores, no cross-core
    communication (weights replicated).
  - bcm is added into the score PSUM by the PE itself via an identity-matmul
    (I.T @ bcm accumulated onto the qk matmul), so the only post-processing is
    a single exp per graph straight out of PSUM.
  - The denominator is obtained free by appending a ones-column to v in the PV
    matmul; the division happens on the HOST (outputs leave the chip
    unnormalized as [num | den] rows in bf16).
  - 1/sqrt(dq) is folded into Wq host-side; everything is pre-cast to bf16.

Layout: q/k projections are computed transposed (d on partitions) straight from
x.T; scores are computed transposed (j on partitions), both 256-col tiles of a
graph sharing one PSUM bank so one exp covers a whole graph. v is computed in
natural layout; its bias lands via a rank-1 ones x bias^T matmul.
"""

import math

import numpy as np
import ml_dtypes

import concourse.bass as bass
import concourse.mybir as mybir
import concourse.tile as tile
from concourse import bacc
from concourse.bass_utils import run_bass_kernel_spmd

# -------- problem constants (hardcoded per spec) --------
N = 4096
DIN = 512
DQ = 128           # == DK
NG = 16            # number of graphs
G = N // NG        # 256 nodes per graph
NCORES = 8
RPC = N // NCORES  # 512 rows per core
GPC = NG // NCORES  # 2 graphs per core
NT = RPC // 128    # 4 row-tiles of 128 per core
KO = DIN // 128    # 4 contraction tiles for the projections
VA = DQ + 1        # v augmented with a ones column (denominator trick)
SCALE = 1.0 / math.sqrt(DQ)
NEG = -200.0       # masked-entry sentinel; exp(-200 + |qk|max) == 0 in bf16
NWARM = 5          # PE HAM warmup matmuls

F32 = mybir.dt.float32
BF16 = mybir.dt.bfloat16

ACT = mybir.ActivationFunctionType
ALU = mybir.AluOpType

BF = ml_dtypes.bfloat16

_CACHE: dict = {}


def build_nc() -> bass.Bass:
    """Build the per-core Bass graph (identical on all 8 cores)."""
    nc = bacc.Bacc(
        "TRN2",
        target_bir_lowering=False,
        debug=False,
        enable_asserts=False,
        num_devices=NCORES,
    )
    xT_d = nc.dram_tensor("xh", [128, KO, RPC], BF16, kind="ExternalInput").ap()
    wqk_d = nc.dram_tensor("wqk", [128, 2, KO, DQ], BF16, kind="ExternalInput").ap()
    wv_d = nc.dram_tensor("wv", [128, KO, DQ], BF16, kind="ExternalInput").ap()
    bia_d = nc.dram_tensor("bias", [DQ, 3], F32, kind="ExternalInput").ap()
    biar_d = nc.dram_tensor("biasr", [1, 3, DQ], BF16, kind="ExternalInput").ap()
    # bcm blocks (transposed, masked) with a 128x128 identity appended
    bc_d = nc.dram_tensor("bch", [128, NT * G + 128], BF16, kind="ExternalInput").ap()
    out_d = nc.dram_tensor("out", [128, NT, VA], BF16, kind="ExternalOutput").ap()

    with tile.TileContext(nc) as tc:
        with (
            tc.tile_pool(name="const", bufs=1) as cpool,
            tc.tile_pool(name="eq", bufs=2) as epool,
            tc.tile_pool(name="ps_proj", bufs=2, space="PSUM") as pp,
            tc.tile_pool(name="ps_v", bufs=2, space="PSUM") as pvp,
            tc.tile_pool(name="ps_s", bufs=2, space="PSUM") as ps,
            tc.tile_pool(name="ps_o", bufs=2, space="PSUM") as po,
        ):
            # ---- input DMAs: wqk + xh first (they gate the projections),
            # small/late tensors behind them. Both HWDGE rings are used.
            wqk = cpool.tile([128, 2, KO, DQ], BF16)
            nc.sync.dma_start(wqk[:], wqk_d)
            xT = cpool.tile([128, KO, RPC], BF16)  # [din%128, din//128, r]
            nc.scalar.dma_start(xT[:, 0:2, :], xT_d[:, 0:2, :])
            wv = cpool.tile([128, KO, DQ], BF16)
            nc.sync.dma_start(wv[:], wv_d)
            nc.scalar.dma_start(xT[:, 2:4, :], xT_d[:, 2:4, :])
            bia = cpool.tile([128, 3], F32)  # [d, qkv]; q column pre-scaled
            nc.sync.dma_start(bia[:], bia_d)
            biar = cpool.tile([1, 3, DQ], BF16)  # row layout for the v bias
            nc.sync.dma_start(biar[:], biar_d)
            bc = cpool.tile([128, NT * G + 128], BF16)  # bcm tiles | identity
            nc.scalar.dma_start(bc[:], bc_d)
            idn = bc[:, NT * G:NT * G + 128]

            # small constants
            warm = cpool.tile([128, RPC], BF16)
            nc.vector.memset(warm[:], 1.0)
            ones_b = cpool.tile([1, 128], BF16)  # rank-1 bias lhsT
            nc.vector.memset(ones_b[:], 1.0)
            vna = cpool.tile([128, NT, VA], BF16)  # [j%128, j//128, d | 1]
            nc.vector.memset(vna[:, :, DQ:VA], 1.0)

            # ---- PE HAM warmup: dummy matmuls while the DMAs stream ----
            for _ in range(NWARM):
                wp = pp.tile([128, RPC], F32, tag="proj")
                nc.tensor.matmul(
                    wp[:], lhsT=warm[:, 0:128], rhs=warm[:],
                    start=True, stop=True,
                )

            # ---- q/k projections, transposed: pT[d, r] = (x @ W_s.T).T ----
            def proj(s):
                p = pp.tile([128, RPC], F32, tag="proj")
                for ko in range(KO):
                    nc.tensor.matmul(
                        p[:],
                        lhsT=wqk[:, s, ko, :],
                        rhs=xT[:, ko, :],
                        start=(ko == 0),
                        stop=(ko == KO - 1),
                    )
                return p

            # evacuate in graph-halves so scores for graph 0 start early;
            # q on DVE, k on ACT to run both engines in parallel
            pq = proj(0)
            qT = cpool.tile([128, RPC], BF16)
            nc.vector.tensor_scalar_add(qT[:, 0:G], pq[:, 0:G], bia[:, 0:1])
            nc.vector.tensor_scalar_add(qT[:, G:RPC], pq[:, G:RPC], bia[:, 0:1])
            pk = proj(1)
            kT = cpool.tile([128, RPC], BF16)
            nc.scalar.activation(kT[:, 0:G], pk[:, 0:G], ACT.Identity, bias=bia[:, 1:2])
            nc.scalar.activation(kT[:, G:RPC], pk[:, G:RPC], ACT.Identity, bias=bia[:, 1:2])

            # ---- v in natural layout (bf16), ones column, rank-1 bias ----
            for jt in range(NT):
                pv = pvp.tile([128, DQ], F32, tag="vn")
                for ko in range(KO):
                    nc.tensor.matmul(
                        pv[:],
                        lhsT=xT[:, ko, jt * 128:(jt + 1) * 128],
                        rhs=wv[:, ko, :],
                        start=(ko == 0),
                        stop=False,
                    )
                nc.tensor.matmul(
                    pv[:], lhsT=ones_b[:], rhs=biar[:, 2, :],
                    start=False, stop=True,
                )
                if jt % 2 == 0:
                    nc.vector.tensor_copy(out=vna[:, jt, 0:DQ], in_=pv[:])
                else:
                    nc.scalar.copy(vna[:, jt, 0:DQ], pv[:])

            # ---- scores: qk into PSUM, bcm added by PE via identity-matmul,
            # one exp per graph straight out of the (single-bank) PSUM tile
            eqs = []
            for g in range(GPC):
                spg = ps.tile([128, 2 * G], F32, tag="s")  # both j-blocks, 1 bank
                # one accumulation group: qk scores overwrite (start clears the
                # bank), then the identity-matmuls accumulate bcm on top
                for jb in range(2):
                    t = 2 * g + jb
                    nc.tensor.matmul(
                        spg[:, jb * G:(jb + 1) * G],
                        lhsT=kT[:, t * 128:(t + 1) * 128],
                        rhs=qT[:, g * G:(g + 1) * G],
                        start=(jb == 0),
                        stop=False,
                        skip_group_check=True,
                    )
                for jb in range(2):
                    t = 2 * g + jb
                    nc.tensor.matmul(
                        spg[:, jb * G:(jb + 1) * G],
                        lhsT=idn,
                        rhs=bc[:, t * G:(t + 1) * G],
                        start=False,
                        stop=(jb == 1),
                        skip_group_check=True,
                    )
                eq = epool.tile([128, 2 * G], BF16, tag="eq")
                nc.scalar.activation(eq[:], spg[:], ACT.Exp)
                eqs.append(eq)

            # ---- PV matmul (+denominator via ones column); the division
            # happens host-side, so just evacuate [num | den] rows
            out_sb = cpool.tile([128, NT, VA], BF16)
            for g in range(GPC):
                for rb in range(2):
                    t = 2 * g + rb
                    op = po.tile([128, VA], F32, tag="o")
                    for jb in range(2):
                        nc.tensor.matmul(
                            op[:],
                            lhsT=eqs[g][:, jb * G + rb * 128: jb * G + rb * 128 + 128],
                            rhs=vna[:, 2 * g + jb, :],
                            start=(jb == 0),
                            stop=(jb == 1),
                        )
                    if rb == 0:
                        nc.vector.tensor_copy(out=out_sb[:, t, :], in_=op[:])
                    else:
                        nc.scalar.copy(out_sb[:, t, :], op[:])
                # per-graph output DMA on alternating rings
                if g == 0:
                    nc.sync.dma_start(out_d[:, 0:2, :], out_sb[:, 0:2, :])
                else:
                    nc.scalar.dma_start(out_d[:, 2:4, :], out_sb[:, 2:4, :])
    nc.compile()
    return nc


def get_nc() -> bass.Bass:
    if "nc" not in _CACHE:
        _CACHE["nc"] = build_nc()
    return _CACHE["nc"]


def make_in_maps(x, b, c, ptr, sparse_mask, Wq, bq, Wk, bk, Wv, bv):
    """Host-side sharding: slice the block-diagonal, combine b+c with the mask
    sentinel, cast everything to bf16, transpose to partition-major layouts."""
    x = np.asarray(x, dtype=np.float32)
    b = np.asarray(b, dtype=np.float32)
    c = np.asarray(c, dtype=np.float32)
    ptr = np.asarray(ptr)
    mask = np.asarray(sparse_mask) != 0
    # fold 1/sqrt(dq) into Wq/bq so scores come out pre-scaled
    wqk3 = np.stack(
        [np.asarray(Wq).T * SCALE, np.asarray(Wk).T]
    ).astype(np.float32)  # [2, DIN, DQ]
    wv3 = np.asarray(Wv).T.astype(np.float32)  # [DIN, DQ]
    bias = np.ascontiguousarray(
        np.stack(
            [np.asarray(bq) * SCALE, np.asarray(bk), np.asarray(bv)], axis=1
        )
    ).astype(np.float32)  # [DQ, 3]
    biasr = np.ascontiguousarray(
        np.stack([np.asarray(bq), np.asarray(bk), np.asarray(bv)], axis=0)[None]
    ).astype(BF)  # [1, 3, DQ]

    assert np.array_equal(
        np.asarray(ptr).ravel(), np.arange(NG + 1) * G
    ), "kernel compiled for uniform 256-node graphs"

    # weights in partition-major layout: wh[p, s, o, d] = W_s.T[o*128+p, d]
    wqk = np.ascontiguousarray(
        wqk3.reshape(2, KO, 128, DQ).transpose(2, 0, 1, 3)
    ).astype(BF)  # [128, 2, KO, DQ]
    wv = np.ascontiguousarray(
        wv3.reshape(KO, 128, DQ).transpose(1, 0, 2)
    ).astype(BF)  # [128, KO, DQ]
    ident = np.eye(128, dtype=np.float32)

    in_maps = []
    for i in range(NCORES):
        lo = i * RPC
        xT = x[lo:lo + RPC].T  # [DIN, RPC]
        xh = np.ascontiguousarray(
            xT.reshape(KO, 128, RPC).transpose(1, 0, 2)
        ).astype(BF)  # [128, KO, RPC]
        bds = []
        for gl in range(GPC):
            blk = slice(lo + gl * G, lo + (gl + 1) * G)
            m = np.where(mask[blk, blk], b[blk, blk] + c[blk, blk], NEG)
            bds.append(m.T)  # [j, r]
        bdT = np.concatenate(bds, 0)  # [RPC, G]
        # bch[p, t*G + r] = bdT[t*128+p, r], identity appended at the end
        bch = np.concatenate(
            [bdT.reshape(NT, 128, G).transpose(1, 0, 2).reshape(128, NT * G),
             ident],
            axis=1,
        )
        bch = np.ascontiguousarray(bch).astype(BF)  # [128, NT*G + 128]
        in_maps.append(
            {"xh": xh, "wqk": wqk, "wv": wv, "bias": bias, "biasr": biasr,
             "bch": bch}
        )
    return in_maps


def run(inputs: dict, trace: bool = False):
    """Run on all 8 cores; returns (full_output, BassKernelResults)."""
    nc = get_nc()
    in_maps = make_in_maps(**inputs)
    res = run_bass_kernel_spmd(
        nc, in_maps, core_ids=list(range(NCORES)), trace=trace
    )
    outs = []
    for r in res.results:
        o = np.asarray(r["out"]).astype(np.float32)  # [128, NT, VA]
        o = o[:, :, 0:DQ] / o[:, :, DQ:VA]  # host-side softmax normalization
        outs.append(o.transpose(1, 0, 2).reshape(RPC, DQ))
    out = np.concatenate(outs, axis=0)
    return out, res


def kernel(**inputs) -> np.ndarray:
    out, _ = run(inputs, trace=False)
    return out
